# revision 4
# baseline (speedup 1.0000x reference)
"""GCNII conv (gnn_message_passing) Trainium2 Bass kernel.

Strategy (8-way node sharding, prematerialized halo-exchanged neighbor
features — per the sharding hint):
  - Host: for each core's node shard, lay the 16 neighbor feature rows plus
    the self row out as 17 channel-major fp8(e4m3) "planes" per node tile
    ([C=128 partitions, 17*nt] per tile, plane-contiguous).  This converts
    the irregular gather into full-bandwidth contiguous streams (512B+ DMA
    descriptors, no SWDGE descriptor-generation cost).
  - Device: per tile, one big fp8 plane load + a bf16 x_0 load; the PE sums
    the 17 planes into PSUM fp32 via fp8 DoubleRow identity matmuls (2
    planes per pass at 0.5 cycles/row); the GCNII combine is two bf16 GEMMs
    (M1 = (s1*I + beta*W1)/deg on the plane sum, M2 = s2*I + beta*W2 on
    x_0) accumulated in PSUM, then bias+ReLU on the activation engine and a
    bf16 store issued from the activation engine's HWDGE queue so stores
    never stall the SP load queue.
  - fp8 is applied only to the aggregated neighbor features (each is 1/17
    of the aggregate); weights, x_0 and the output stay bf16.  Measured
    end-to-end rel err ~8e-3, well under the 2e-2 gate.
"""

import numpy as np
import ml_dtypes

import concourse.bacc as bacc
import concourse.mybir as mybir
from concourse.tile import TileContext
from concourse.bass_utils import run_bass_kernel_spmd

BF16 = ml_dtypes.bfloat16
F8 = ml_dtypes.float8_e4m3
F32 = np.float32

ALPHA = 0.1
BETA = float(np.log(0.5 / 4 + 1.0))
DEG_K = 16           # neighbors per node (w/o self loop)
C = 128              # channels
P = 128              # partitions
PLANES = DEG_K + 1   # neighbor planes + self plane

N_FULL = 40000
N_CORES = 8
NSH = N_FULL // N_CORES

WORK_BUFS = 3
PSUM_BUFS = 4


def _split_tiles(nsh):
    """512-node tiles with a tapered tail: the last (small) tile bounds the
    exposed end-of-kernel load->compute->store chain."""
    n_full, left = divmod(nsh, 512)
    if left == 0:
        tail = []
    elif left > 256:
        tail = [left - 128, 128]
    else:
        tail = [left]
    tiles = [512] * n_full + tail
    assert sum(tiles) == nsh
    return tiles


# --------------------------------------------------------------------------
# device program
# --------------------------------------------------------------------------

def _build_program(meta):
    nsh = meta["nsh"]
    tiles = meta["tiles"]
    dt = mybir.dt
    nc = bacc.Bacc("TRN2", target_bir_lowering=False)

    planes_d = nc.dram_tensor("planes", [P, PLANES * nsh], dt.float8e4,
                              kind="ExternalInput")
    x0t_d = nc.dram_tensor("x0t", [P, nsh], dt.bfloat16, kind="ExternalInput")
    m1t_d = nc.dram_tensor("m1t", [P, C], dt.bfloat16, kind="ExternalInput")
    m2t_d = nc.dram_tensor("m2t", [P, C], dt.bfloat16, kind="ExternalInput")
    bias_d = nc.dram_tensor("biasv", [P, 1], dt.float32, kind="ExternalInput")
    idd_d = nc.dram_tensor("idd", [P, 2 * C], dt.float8e4, kind="ExternalInput")
    ids_d = nc.dram_tensor("ids", [P, C], dt.float8e4, kind="ExternalInput")
    out_d = nc.dram_tensor("out", [P, nsh], dt.bfloat16, kind="ExternalOutput")

    hA = 8          # planes in the first load chunk
    hB = PLANES - hA
    tile_off = np.cumsum([0] + list(tiles))

    with TileContext(nc) as tc:
        with (
            tc.tile_pool(name="consts", bufs=1) as cpool,
            tc.tile_pool(name="work", bufs=WORK_BUFS) as pool,
            tc.tile_pool(name="psum", bufs=PSUM_BUFS, space="PSUM") as ppool,
        ):
            def load_tile(t):
                nt = tiles[t]
                off = int(tile_off[t])
                base = PLANES * off
                g_a = pool.tile([P, hA, nt], dt.float8e4, name="g_a")
                nc.sync.dma_start(
                    out=g_a[:], in_=planes_d[:, base:base + hA * nt])
                g_b = pool.tile([P, hB, nt], dt.float8e4, name="g_b")
                nc.sync.dma_start(
                    out=g_b[:],
                    in_=planes_d[:, base + hA * nt:base + PLANES * nt])
                x0_t = pool.tile([P, nt], dt.bfloat16, name="x0")
                nc.sync.dma_start(out=x0_t[:], in_=x0t_d[:, off:off + nt])
                return g_a, g_b, x0_t

            # tile-0 loads first so the DMA engines start on the critical
            # stream immediately; small consts ride the Activation queue
            pre = load_tile(0)

            m1t = cpool.tile([P, C], dt.bfloat16)
            nc.scalar.dma_start(out=m1t[:], in_=m1t_d[:])
            m2t = cpool.tile([P, C], dt.bfloat16)
            nc.scalar.dma_start(out=m2t[:], in_=m2t_d[:])
            biasv = cpool.tile([P, 1], dt.float32)
            nc.scalar.dma_start(out=biasv[:], in_=bias_d[:])
            idd = cpool.tile([P, 2, C], dt.float8e4)
            nc.scalar.dma_start(out=idd[:], in_=idd_d[:])
            ids = cpool.tile([P, C], dt.float8e4)
            nc.scalar.dma_start(out=ids[:], in_=ids_d[:])

            for t, nt in enumerate(tiles):
                off = int(tile_off[t])
                g_a, g_b, x0_t = pre if t == 0 else load_tile(t)

                psum_a = ppool.tile([P, nt], dt.float32)
                for j in range(hA // 2):
                    nc.tensor.matmul(
                        psum_a[:], lhsT=idd[:], rhs=g_a[:, 2 * j:2 * j + 2, :],
                        start=(j == 0), stop=False,
                        perf_mode=mybir.MatmulPerfMode.DoubleRow)
                for j in range((hB - 1) // 2):
                    nc.tensor.matmul(
                        psum_a[:], lhsT=idd[:], rhs=g_b[:, 2 * j:2 * j + 2, :],
                        start=False, stop=False,
                        perf_mode=mybir.MatmulPerfMode.DoubleRow)
                nc.tensor.matmul(psum_a[:], lhsT=ids[:], rhs=g_b[:, hB - 1, :],
                                 start=False, stop=True)

                gsum = pool.tile([P, nt], dt.bfloat16, name="gsum")
                nc.vector.tensor_copy(out=gsum[:], in_=psum_a[:])

                psum_b = ppool.tile([P, nt], dt.float32)
                nc.tensor.matmul(psum_b[:], lhsT=m1t[:], rhs=gsum[:],
                                 start=True, stop=False)
                nc.tensor.matmul(psum_b[:], lhsT=m2t[:], rhs=x0_t[:],
                                 start=False, stop=True)

                out_t = pool.tile([P, nt], dt.bfloat16, name="out")
                nc.scalar.activation(
                    out_t[:], psum_b[:], mybir.ActivationFunctionType.Relu,
                    bias=biasv[:, 0:1], scale=1.0)
                nc.scalar.dma_start(out=out_d[:, off:off + nt], in_=out_t[:])
    nc.compile()
    return nc


# --------------------------------------------------------------------------
# host prep
# --------------------------------------------------------------------------

def _prepare(x, x_0, edge_index, W1, W2, bias):
    x = np.asarray(x, dtype=F32)          # [1, C, N, 1]
    x_0 = np.asarray(x_0, dtype=F32)      # [1, N, C]
    ei = np.asarray(edge_index)           # [2, 1, N, K]
    W1 = np.asarray(W1, dtype=F32)
    W2 = np.asarray(W2, dtype=F32)
    bias = np.asarray(bias, dtype=F32)

    n_rows = x.shape[2]
    assert n_rows == N_FULL
    nsh = n_rows // N_CORES
    idx_all = np.asarray(ei[0, 0], dtype=np.int64)   # [N, K]
    assert idx_all.shape[1] == DEG_K

    x8_nc = np.ascontiguousarray(x[0, :, :, 0].T).astype(F8)   # [N, C]
    x0_cn = np.ascontiguousarray(x_0[0].T)                     # [C, N]

    deg = DEG_K + 1
    s1 = (1.0 - ALPHA) * (1.0 - BETA)
    s2 = ALPHA * (1.0 - BETA)
    eye = np.eye(C, dtype=np.float64)
    m1t = ((s1 * eye + BETA * W1.astype(np.float64)).T / deg).astype(BF16)
    m2t = ((s2 * eye + BETA * W2.astype(np.float64)).T).astype(BF16)
    bias_v = np.ascontiguousarray(bias.reshape(-1)[:, None].astype(F32))
    idd = np.zeros((P, 2 * C), dtype=F8)
    idd[np.arange(P), np.arange(P)] = 1.0
    idd[np.arange(P), C + np.arange(P)] = 1.0
    ids = np.eye(P, dtype=F8)

    tiles = _split_tiles(nsh)

    in_maps = []
    for c in range(N_CORES):
        sl = slice(c * nsh, (c + 1) * nsh)
        idxs = idx_all[sl]                               # [nsh, K]
        planes = np.empty((PLANES, nsh, C), dtype=F8)
        for k in range(DEG_K):
            planes[k] = x8_nc[idxs[:, k]]
        planes[DEG_K] = x8_nc[sl]
        cols = np.empty((C, PLANES * nsh), dtype=F8)
        off = 0
        for nt in tiles:
            blk = planes[:, off:off + nt, :]             # [17, nt, C]
            cols[:, PLANES * off:PLANES * (off + nt)] = (
                blk.transpose(2, 0, 1).reshape(C, PLANES * nt))
            off += nt
        in_maps.append(dict(
            planes=cols,
            x0t=np.ascontiguousarray(x0_cn[:, sl]).astype(BF16),
            m1t=m1t,
            m2t=m2t,
            biasv=bias_v,
            idd=idd,
            ids=ids,
        ))
    meta = dict(nsh=nsh, tiles=tiles)
    return in_maps, meta


last_results = None  # BassKernelResults of the most recent kernel() call


def kernel(x, x_0, edge_index, W1, W2, bias):
    global last_results
    import os
    in_maps, meta = _prepare(x, x_0, edge_index, W1, W2, bias)
    nc = _build_program(meta)
    trace = os.environ.get("GCNII_TRACE", "") == "1"
    res = run_bass_kernel_spmd(nc, in_maps, core_ids=list(range(N_CORES)),
                               trace=trace)
    last_results = res
    out = np.concatenate([r["out"] for r in res.results], axis=1)
    return np.ascontiguousarray(out.astype(F32))[None, :, :, None]


# revision 17
# speedup vs baseline: 1.1778x; 1.1778x over previous
"""GCNII conv (gnn_message_passing) Trainium2 Bass kernel.

Strategy (8-way node sharding, prematerialized halo-exchanged neighbor
features — per the sharding hint):
  - Host: for each core's node shard, lay out 18 channel-major fp8(e4m3)
    "planes" per node tile: the 16 neighbor feature rows, the self row, and
    the x_0 row ([C=128 partitions, 18*nt] per tile, plane-contiguous).
    This converts the irregular gather into full-bandwidth contiguous
    streams (512B+ DMA descriptors, no SWDGE descriptor-generation cost).
  - Device: per tile, two fp8 plane loads (split so compute can start on
    the first chunk); the PE applies the GCNII combine directly with fp8
    DoubleRow matmuls (2 planes per pass at 0.5 cycles/row): lhsT is
    [M1s|M1s] for neighbor/self pairs and [M1s|M2s] for the (self, x_0)
    pair, where M1s = 64*(s1*I + beta*W1)/deg and M2s = 64*(s2*I +
    beta*W2).  The 2^6 scale keeps all fp8 weight entries in the normal
    range; the activation stage rescales by 2^-6 while applying bias+ReLU,
    then the bf16 store issues from the Pool engine's SWDGE queue so
    stores never stall the SP load queue or the activation engine.
  - fp8 affects only terms that are each a small fraction of the output;
    measured end-to-end rel err ~1.1e-2, under the 2e-2 gate.
"""

import numpy as np
import ml_dtypes

import concourse.bacc as bacc
import concourse.mybir as mybir
from concourse.tile import TileContext
from concourse.bass_utils import run_bass_kernel_spmd

BF16 = ml_dtypes.bfloat16
F8 = ml_dtypes.float8_e4m3
F32 = np.float32

ALPHA = 0.1
BETA = float(np.log(0.5 / 4 + 1.0))
DEG_K = 16           # neighbors per node (w/o self loop)
C = 128              # channels
P = 128              # partitions
PLANES = DEG_K + 2   # neighbor planes + self plane + x0 plane
WSCALE = 6           # weights pre-scaled by 2**WSCALE (act rescales back)

N_FULL = 40000
N_CORES = 8
NSH = N_FULL // N_CORES

WORK_BUFS = 4
PSUM_BUFS = 3
H_A = 12             # planes in the first load chunk (even), rest in chunk 2
CONST_BYTES = 516    # m1dd 256 | m12dd 256 | bias 4


def _split_tiles(nsh):
    """512-node tiles with one smaller tail tile: the small last tile bounds
    the exposed end-of-kernel load->compute->store chain."""
    n_full, left = divmod(nsh, 512)
    tiles = [512] * n_full + ([left] if left else [])
    assert sum(tiles) == nsh
    return tiles


# --------------------------------------------------------------------------
# device program
# --------------------------------------------------------------------------

def _build_program(meta):
    nsh = meta["nsh"]
    tiles = meta["tiles"]
    dt = mybir.dt
    nc = bacc.Bacc("TRN2", target_bir_lowering=False)

    planes_d = nc.dram_tensor("planes", [P, PLANES * nsh], dt.float8e4,
                              kind="ExternalInput")
    consts_d = nc.dram_tensor("consts", [P, CONST_BYTES], dt.uint8,
                              kind="ExternalInput")
    out_d = nc.dram_tensor("out", [P, nsh], dt.bfloat16, kind="ExternalOutput")

    hA = H_A
    hB = PLANES - hA
    assert hA % 2 == 0 and hB % 2 == 0
    tile_off = np.cumsum([0] + list(tiles))

    with TileContext(nc) as tc:
        with (
            tc.tile_pool(name="consts", bufs=1) as cpool,
            tc.tile_pool(name="work", bufs=WORK_BUFS) as pool,
            tc.tile_pool(name="psum", bufs=PSUM_BUFS, space="PSUM") as ppool,
        ):
            # one merged const load first (its transfer is ~360 ns), then the
            # tile-0 plane stream — PE can start as soon as g_a(0) lands
            blob = cpool.tile([P, CONST_BYTES], dt.uint8)
            nc.scalar.dma_start(out=blob[:], in_=consts_d[:])
            m1dd = blob[:, 0:256].bitcast(dt.float8e4).rearrange(
                "p (a b) -> p a b", a=2)
            m12dd = blob[:, 256:512].bitcast(dt.float8e4).rearrange(
                "p (a b) -> p a b", a=2)
            biasv = blob[:, 512:516].bitcast(dt.float32)

            def load_tile(t):
                nt = tiles[t]
                base = PLANES * int(tile_off[t])
                g_a = pool.tile([P, hA, nt], dt.float8e4, name="g_a")
                nc.sync.dma_start(
                    out=g_a[:], in_=planes_d[:, base:base + hA * nt])
                g_b = pool.tile([P, hB, nt], dt.float8e4, name="g_b")
                nc.sync.dma_start(
                    out=g_b[:],
                    in_=planes_d[:, base + hA * nt:base + PLANES * nt])
                return g_a, g_b

            pre = load_tile(0)

            for t, nt in enumerate(tiles):
                off = int(tile_off[t])
                g_a, g_b = pre if t == 0 else load_tile(t)

                psum = ppool.tile([P, nt], dt.float32)
                for j in range(hA // 2):
                    nc.tensor.matmul(
                        psum[:], lhsT=m1dd, rhs=g_a[:, 2 * j:2 * j + 2, :],
                        start=(j == 0), stop=False,
                        perf_mode=mybir.MatmulPerfMode.DoubleRow)
                for j in range(hB // 2 - 1):
                    nc.tensor.matmul(
                        psum[:], lhsT=m1dd, rhs=g_b[:, 2 * j:2 * j + 2, :],
                        start=False, stop=False,
                        perf_mode=mybir.MatmulPerfMode.DoubleRow)
                nc.tensor.matmul(
                    psum[:], lhsT=m12dd, rhs=g_b[:, hB - 2:hB, :],
                    start=False, stop=True,
                    perf_mode=mybir.MatmulPerfMode.DoubleRow)

                out_t = pool.tile([P, nt], dt.bfloat16, name="out")
                nc.scalar.activation(
                    out_t[:], psum[:], mybir.ActivationFunctionType.Relu,
                    bias=biasv[:, 0:1], scale=2.0 ** -WSCALE)
                # steady-state stores ride the idle Pool SWDGE queue; the
                # last two run on SP HWDGE (shorter pipe on the exposed tail,
                # and the SP load queue is empty by then)
                store_eng = nc.sync if t >= len(tiles) - 2 else nc.gpsimd
                store_eng.dma_start(out=out_d[:, off:off + nt], in_=out_t[:])
    nc.compile()
    return nc


# --------------------------------------------------------------------------
# host prep
# --------------------------------------------------------------------------

def _prepare(x, x_0, edge_index, W1, W2, bias):
    x = np.asarray(x, dtype=F32)          # [1, C, N, 1]
    x_0 = np.asarray(x_0, dtype=F32)      # [1, N, C]
    ei = np.asarray(edge_index)           # [2, 1, N, K]
    W1 = np.asarray(W1, dtype=F32)
    W2 = np.asarray(W2, dtype=F32)
    bias = np.asarray(bias, dtype=F32)

    n_rows = x.shape[2]
    assert n_rows == N_FULL
    nsh = n_rows // N_CORES
    idx_all = np.asarray(ei[0, 0], dtype=np.int64)   # [N, K]
    assert idx_all.shape[1] == DEG_K

    x8_nc = np.ascontiguousarray(x[0, :, :, 0].T).astype(F8)   # [N, C]
    x08_nc = np.ascontiguousarray(x_0[0]).astype(F8)           # [N, C]

    deg = DEG_K + 1
    s1 = (1.0 - ALPHA) * (1.0 - BETA)
    s2 = ALPHA * (1.0 - BETA)
    eye = np.eye(C, dtype=np.float64)
    sc = float(2.0 ** WSCALE)
    m1s = ((s1 * eye + BETA * W1.astype(np.float64)).T / deg * sc).astype(F8)
    m2s = ((s2 * eye + BETA * W2.astype(np.float64)).T * sc).astype(F8)
    bias_v = np.ascontiguousarray(bias.reshape(-1)[:, None].astype(F32))
    m1dd = np.concatenate([m1s, m1s], axis=1)        # [P, 2C]
    m12dd = np.concatenate([m1s, m2s], axis=1)       # [P, 2C]
    consts = np.concatenate([
        np.ascontiguousarray(m1dd).view(np.uint8),
        np.ascontiguousarray(m12dd).view(np.uint8),
        bias_v.view(np.uint8),
    ], axis=1)
    assert consts.shape == (P, CONST_BYTES), consts.shape

    tiles = _split_tiles(nsh)

    in_maps = []
    for c in range(N_CORES):
        sl = slice(c * nsh, (c + 1) * nsh)
        idxs = idx_all[sl]                               # [nsh, K]
        planes = np.empty((PLANES, nsh, C), dtype=F8)
        for k in range(DEG_K):
            planes[k] = x8_nc[idxs[:, k]]
        planes[DEG_K] = x8_nc[sl]
        planes[DEG_K + 1] = x08_nc[sl]
        cols = np.empty((C, PLANES * nsh), dtype=F8)
        off = 0
        for nt in tiles:
            blk = planes[:, off:off + nt, :]             # [18, nt, C]
            cols[:, PLANES * off:PLANES * (off + nt)] = (
                blk.transpose(2, 0, 1).reshape(C, PLANES * nt))
            off += nt
        in_maps.append(dict(planes=cols, consts=consts))
    meta = dict(nsh=nsh, tiles=tiles)
    return in_maps, meta


last_results = None  # BassKernelResults of the most recent kernel() call


def kernel(x, x_0, edge_index, W1, W2, bias):
    global last_results
    import os
    in_maps, meta = _prepare(x, x_0, edge_index, W1, W2, bias)
    nc = _build_program(meta)
    trace = os.environ.get("GCNII_TRACE", "") == "1"
    res = run_bass_kernel_spmd(nc, in_maps, core_ids=list(range(N_CORES)),
                               trace=trace)
    last_results = res
    out = np.concatenate([r["out"] for r in res.results], axis=1)
    return np.ascontiguousarray(out.astype(F32))[None, :, :, None]
